# revision 4
# baseline (speedup 1.0000x reference)
"""Trainium2 Bass kernel for nn_BuildK — instruction-count-minimal rewrite.

Shard y across 8 cores (no cross-core comms). Per core, x=128 on partitions,
single full-volume chunk (z=64, y=16+2 halo). Selection of the 9
intensity-nearest of 27 neighbors uses exact f32 |diff| keys through a
liveness-pruned top-8 sorting network; sorted values / per-rank neighbor
weights are recovered with equality masks + broadcast APs. Multi-lane
(overlapping-stride) views batch the oy-neighbor triples into single
instructions wherever the emulator's per-instruction cost dominates.
"""

import sys

sys.path.insert(0, "/opt/trn_rl_repo")

import numpy as np

H, M, N = 64, 128, 128
NCORES = 8
YS = M // NCORES          # 16 owned y rows per core
YE = YS + 2               # 18 = ext region (owned + 1 halo each side)
YI = YS + 4               # 20 = input slab y extent (halo 2)
ZE = H + 2                # 66 = z extent with periodic wrap rows
KN = 9
EPS = 1e-6
NSLOT = 22                # wire slots for the selection network
FD = H * YE               # 1152 ext-region elems

OFFS = [(oz, oy, ox) for oz in (-1, 0, 1) for oy in (-1, 0, 1)
        for ox in (-1, 0, 1)]            # reference enumeration; 13 = center

# wire -> candidate d, arranged so each sorter group is fed by oy-runs
# (fixed oz, ox; d stride 3) that become multi-lane views:
#   sorter A (wires 0-8):  oz=-1 block, 3 triples
#   sorter B (wires 9-17): oz=+1 block, 3 triples
#   sorter C (wires 18-25): oz=0 block, 2 triples + the (oy=+-1, ox=0) pair
WIRE_CAND = [0, 3, 6, 1, 4, 7, 2, 5, 8,
             18, 21, 24, 19, 22, 25, 20, 23, 26,
             9, 12, 15, 11, 14, 17, 10, 16]
KEY_RUNS = [(0, 3, 0), (3, 3, 1), (6, 3, 2),
            (9, 3, 18), (12, 3, 19), (15, 3, 20),
            (18, 3, 9), (21, 3, 11), (24, 2, 10)]  # (wire0, n, d0)
D_RUNS = [(3, 0), (3, 1), (3, 2), (3, 18), (3, 19), (3, 20),
          (3, 9), (3, 11), (2, 10)]                # (n, d0) for recon


def _lane_stride(d0, n):
    return 2 if (n == 2) else 1   # the pair run is oy=-1,+1 (step 2)


# --------------------------------------------------------------------------
# Selection network: sorted top-8 of the 26 non-center candidates.
# --------------------------------------------------------------------------

_SORT9 = [(0, 3), (1, 7), (2, 5), (4, 8), (0, 7), (2, 4), (3, 8), (5, 6),
          (0, 2), (1, 3), (4, 5), (7, 8), (1, 4), (3, 6), (5, 7), (0, 1),
          (2, 4), (3, 5), (6, 8), (2, 3), (4, 5), (6, 7), (1, 2), (3, 4),
          (5, 6)]

_S8 = [(0, 1), (2, 3), (4, 5), (6, 7), (0, 2), (1, 3), (4, 6), (5, 7),
       (1, 2), (5, 6), (0, 4), (3, 7), (1, 5), (2, 6), (1, 4), (3, 6),
       (2, 4), (3, 5), (3, 4)]


def _oddeven_merge(lo, n, r, out):
    step = r * 2
    if step < n:
        _oddeven_merge(lo, n, step, out)
        _oddeven_merge(lo + r, n, step, out)
        for i in range(lo + r, lo + n - r, step):
            out.append((i, i + r))
    else:
        out.append((lo, lo + r))


def _merge_topk(lenA, lenB, k):
    ces = []
    _oddeven_merge(0, 32, 1, ces)
    inf = [False] * 32
    for w in range(lenA, 16):
        inf[w] = True
    for w in range(16 + lenB, 32):
        inf[w] = True
    label = list(range(32))
    kept = []
    for (i, j) in ces:
        if inf[i] and inf[j]:
            continue
        if inf[j] and not inf[i]:
            continue
        if inf[i] and not inf[j]:
            label[i], label[j] = label[j], label[i]
            inf[i], inf[j] = False, True
            continue
        kept.append((label[i], label[j]))
    needed = set(label[w] for w in range(k))
    keep = []
    for (i, j) in reversed(kept):
        if i in needed or j in needed:
            keep.append((i, j))
            needed.add(i)
            needed.add(j)
    keep.reverse()

    def rm(w):
        return w if w < 16 else w - 16 + lenA

    return [(rm(i), rm(j)) for (i, j) in keep], [rm(label[w]) for w in range(k)]


def build_plan():
    """Ordered ops: ("keyg", run_idx) | ("ce", i, j, ni, nj); outw; lastw."""
    raw = []
    raw += [("keyg", 0), ("keyg", 1), ("keyg", 2)]
    raw += [("ce", i, j) for (i, j) in _SORT9]
    raw += [("keyg", 3), ("keyg", 4), ("keyg", 5)]
    raw += [("ce", i + 9, j + 9) for (i, j) in _SORT9]
    m1, ow1 = _merge_topk(9, 9, 8)
    raw += [("ce", i, j) for (i, j) in m1]
    raw += [("keyg", 6), ("keyg", 7), ("keyg", 8)]
    raw += [("ce", i + 18, j + 18) for (i, j) in _S8]
    m2, ow2 = _merge_topk(8, 8, 8)
    remap = {i: ow1[i] for i in range(8)}
    remap.update({8 + i: 18 + i for i in range(8)})
    raw += [("ce", remap[i], remap[j]) for (i, j) in m2]
    outw = [remap[w] for w in ow2]

    live = set(outw)
    ops = []
    for op_ in reversed(raw):
        if op_[0] == "keyg":
            ops.append(op_)
            continue
        _, i, j = op_
        ni, nj = i in live, j in live
        if not (ni or nj):
            continue
        ops.append(("ce", i, j, ni, nj))
        live.add(i)
        live.add(j)
    ops.reverse()

    lastw = {}
    for t, op_ in enumerate(ops):
        if op_[0] == "keyg":
            w0, n, _ = KEY_RUNS[op_[1]]
            for w in range(w0, w0 + n):
                lastw[w] = t
        else:
            _, i, j, ni, nj = op_
            if ni:
                lastw[i] = t
            if nj:
                lastw[j] = t
    for r, w in enumerate(outw):
        assert PLANCHK(ops, lastw, w), "output wire last-written by key op"
    return ops, outw, lastw


def PLANCHK(ops, lastw, w):
    return ops[lastw[w]][0] == "ce"


PLAN_OPS, PLAN_OUTW, PLAN_LASTW = build_plan()


# --------------------------------------------------------------------------
# Bass graph
# --------------------------------------------------------------------------

def build_bass(ks_value: float, reps: int = 1):
    import bass_rust
    from concourse import bacc, mybir
    from concourse import tile
    from concourse.alu_op_type import AluOpType as op
    from concourse.bass import MemorySpace

    f32 = mybir.dt.float32
    f16 = mybir.dt.float16
    u8 = mybir.dt.uint8
    AF = mybir.ActivationFunctionType

    nc = bacc.Bacc("TRN2", target_bir_lowering=False, debug=False,
                   num_devices=NCORES)

    xin = nc.dram_tensor("xin", [128, 3, ZE, YI], f32, kind="ExternalInput").ap()
    outd = nc.dram_tensor("out", [128, H, YS, KN], f32,
                          kind="ExternalOutput").ap()

    dve = nc.vector
    act = nc.scalar

    rank_of = {w: r for r, w in enumerate(PLAN_OUTW)}  # wire -> K8 column

    def lanes(ap3, n, stride):
        """Insert a leading free dim [n, stride] into a [128, a, b] AP."""
        b = ap3.copy()
        pairs = [list(p) for p in ap3.ap]
        b.ap = bass_rust.VecI64Pair([pairs[0], [stride, n]] + pairs[1:])
        return b

    with tile.TileContext(nc) as tc:
      for _rep in range(reps):
        with tc.tile_pool(name="pp", bufs=1) as pp:
            X3 = pp.tile([128, 3, ZE, YI], f32, tag="X3")
            W = pp.tile([128, KN, ZE, YE], f32, tag="W")
            nc.sync.dma_start(out=X3[:], in_=xin[:])

            # ext-region views (z rows 0..63, y -1..16)
            def vview(d):
                oz, oy, ox = OFFS[d]
                return X3[:, ox + 1, 1 + oz:65 + oz, 1 + oy:19 + oy]

            def vrun(d0, n):
                """Multi-lane view: lanes l=0..n-1 are candidates d0+3l
                (same oz/ox, oy ascending)."""
                stride = _lane_stride(d0, n)
                return lanes(vview(d0), n, stride)

            cv = X3[:, 1, 1:65, 1:19]

            with tc.tile_pool(name="kp", bufs=1) as kp:
                K8 = kp.tile([128, 8, H, YE], f32, tag="K8")

                # ------------- keys + selection network -------------
                with tc.tile_pool(name="sp", bufs=1) as sp:
                    kbig = sp.tile([128, NSLOT, FD], f32, tag="kbig")
                    free_slots = list(range(NSLOT))
                    wire_ap = {}
                    wire_slot = {}

                    def alloc_ap(wire, t):
                        if t == PLAN_LASTW[wire] and wire in rank_of:
                            return K8[:, rank_of[wire]], None
                        s = free_slots.pop()
                        return kbig[:, s, :], s

                    def alloc_run(n):
                        ss = sorted(free_slots)
                        for i in range(len(ss) - n + 1):
                            if ss[i + n - 1] == ss[i] + n - 1:
                                for s in range(ss[i], ss[i] + n):
                                    free_slots.remove(s)
                                return ss[i]
                        raise RuntimeError("no contiguous slot run")

                    for t, op_ in enumerate(PLAN_OPS):
                        if op_[0] == "keyg":
                            w0, n, d0 = KEY_RUNS[op_[1]]
                            s0 = alloc_run(n)
                            kv = kbig[:, s0:s0 + n, :]
                            dve.tensor_tensor(
                                out=kv, in0=vrun(d0, n),
                                in1=cv.unsqueeze(1).to_broadcast(
                                    [128, n, H, YE]), op=op.subtract)
                            dve.scalar_tensor_tensor(
                                out=kv, in0=kv, scalar=-1.0, in1=kv,
                                op0=op.mult, op1=op.max)
                            for l in range(n):
                                wire_ap[w0 + l] = kbig[:, s0 + l, :]
                                wire_slot[w0 + l] = s0 + l
                            continue
                        _, i, j, ni, nj = op_
                        ai, aj = wire_ap[i], wire_ap[j]
                        si, sj = wire_slot[i], wire_slot[j]
                        if ni:
                            new_ai, new_si = alloc_ap(i, t)
                            dve.tensor_tensor(out=new_ai, in0=ai, in1=aj,
                                              op=op.min)
                        if nj:
                            new_aj, new_sj = alloc_ap(j, t)
                            dve.tensor_tensor(out=new_aj, in0=ai, in1=aj,
                                              op=op.max)
                        if si is not None:
                            free_slots.append(si)
                        if sj is not None:
                            free_slots.append(sj)
                        if ni:
                            wire_ap[i], wire_slot[i] = new_ai, new_si
                        else:
                            del wire_ap[i], wire_slot[i]
                        if nj:
                            wire_ap[j], wire_slot[j] = new_aj, new_sj
                        else:
                            del wire_ap[j], wire_slot[j]

                # ------------- W reconstruction (equality masks) ------------
                with tc.tile_pool(name="cp", bufs=1) as cp:
                    ktg = cp.tile([128, 3, H, YE], f32, tag="ktg")
                    mg = cp.tile([128, 3, 8, FD], u8, tag="mg")
                    K8f = K8[:].rearrange("p r a b -> p r (a b)")
                    for (n, d0) in D_RUNS:
                        kv = ktg[:, 0:n]
                        dve.tensor_tensor(
                            out=kv, in0=vrun(d0, n),
                            in1=cv.unsqueeze(1).to_broadcast(
                                [128, n, H, YE]), op=op.subtract)
                        dve.scalar_tensor_tensor(
                            out=kv, in0=kv, scalar=-1.0, in1=kv,
                            op0=op.mult, op1=op.max)
                        kvf = kv.rearrange("p n a b -> p n (a b)")
                        dve.tensor_tensor(
                            out=mg[:, 0:n],
                            in0=K8f.unsqueeze(1).to_broadcast(
                                [128, n, 8, FD]),
                            in1=kvf.unsqueeze(2).to_broadcast(
                                [128, n, 8, FD]),
                            op=op.is_equal)
                        stride = _lane_stride(d0, n)
                        for l in range(n):
                            d = d0 + 3 * stride * l
                            dve.select(W[:, 1:KN, 1:65, :], mg[:, l],
                                       vview(d).unsqueeze(1).to_broadcast(
                                           [128, 8, H, YE]),
                                       W[:, 1:KN, 1:65, :])
                    act.activation(out=W[:, 0, 1:65, :], in_=cv, func=AF.Copy)

            # z wrap rows of W
            nc.sync.dma_start(out=W[:, :, 0:1, :], in_=W[:, :, 64:65, :])
            nc.sync.dma_start(out=W[:, :, 65:66, :], in_=W[:, :, 1:2, :])

            # ------------- dots + softmax -------------
            with tc.tile_pool(name="dp", bufs=1) as dp:
                wr = dp.tile([128, KN, ZE, YE], f32, tag="wr")
                Qe = dp.tile([128, ZE, YE], f32, tag="Qe")
                Qr = dp.tile([128, ZE, YE], f32, tag="Qr")
                Se = dp.tile([128, ZE, YE], f32, tag="Se")
                P = dp.tile([128, H, YS], f32, tag="P")
                scv = dp.tile([128, H, YS], f32, tag="scv")
                esel = dp.tile([128, KN, H, YS], f16, tag="esel")

                with tc.tile_pool(name="dp1", bufs=1) as dp1:
                    prod = dp1.tile([128, H, YS, KN], f32, tag="prod")
                    m8 = dp1.tile([128, 8, H, YS], u8, tag="m8")
                    Db = dp1.tile([128, 3, H, YS], f32, tag="Db")
                    # etb aliases the front of prod (free there once the
                    # last reduce of a row has consumed prod)
                    etb = prod[:].rearrange("p a b r -> p (a b r)")[
                        :, 0:3 * H * YS].rearrange(
                        "p (l a b) -> p l a b", l=3, a=H, b=YS)

                    # Q (sum of squares) and S (sum) over ranks, ext region,
                    # chunked through the owned-size prod scratch
                    pflat = prod[:].rearrange("p a b r -> p (a b) r")
                    for h in range(2):
                        zlo = 33 * h
                        sl = 33 * YE
                        pv = pflat[:, 0:sl, :].transpose([0, 2, 1])
                        wv = W[:, :, zlo:zlo + 33, :].rearrange(
                            "p r a b -> p r (a b)")
                        qv = Qe[:, zlo:zlo + 33, :].rearrange(
                            "p a b -> p (a b)")
                        sv = Se[:, zlo:zlo + 33, :].rearrange(
                            "p a b -> p (a b)")
                        dve.tensor_tensor(out=pv, in0=wv, in1=wv, op=op.mult)
                        dve.tensor_reduce(out=qv, in_=pflat[:, 0:sl, :],
                                          axis=mybir.AxisListType.X,
                                          op=op.add)
                        act.activation(out=pv, in_=wv, func=AF.Copy)
                        dve.tensor_reduce(out=sv, in_=pflat[:, 0:sl, :],
                                          axis=mybir.AxisListType.X,
                                          op=op.add)

                    # sigma / scale planes (owned region); etb = scratch
                    Qo = Qe[:, 1:65, 1:17]
                    S1 = Se[:, 1:65, 1:17]
                    ta, tb = etb[:, 0], etb[:, 1]
                    act.activation(out=ta, in_=S1, func=AF.Square)
                    dve.scalar_tensor_tensor(out=ta, in0=ta,
                                             scalar=-1.0 / 9.0, in1=Qo,
                                             op0=op.mult, op1=op.add)
                    dve.tensor_scalar(out=tb, in0=ta, scalar1=0.0,
                                      scalar2=None, op0=op.is_equal)
                    dve.tensor_tensor(out=tb, in0=tb, in1=ta, op=op.add)
                    dve.reciprocal(out=scv[:], in_=tb)
                    dve.tensor_scalar(out=scv[:], in0=scv[:],
                                      scalar1=-4.0 / (ks_value * ks_value),
                                      scalar2=None, op0=op.mult)
                    dve.tensor_scalar(out=tb, in0=ta, scalar1=0.0,
                                      scalar2=None, op0=op.not_equal)
                    dve.tensor_tensor(out=scv[:], in0=scv[:], in1=tb,
                                      op=op.mult)
                    dve.scalar_tensor_tensor(out=P[:], in0=S1,
                                             scalar=2.0 * EPS, in1=Qo,
                                             op0=op.mult, op1=op.add)
                    dve.tensor_scalar(out=P[:], in0=P[:],
                                      scalar1=9.0 * EPS * EPS, scalar2=None,
                                      op0=op.add)
                    dve.tensor_tensor(out=P[:], in0=P[:], in1=scv[:],
                                      op=op.mult)
                    # fold eps into neighbor plane: Qe <- Qe - 2 eps Se
                    dve.scalar_tensor_tensor(out=Qe[:], in0=Se[:],
                                             scalar=-2.0 * EPS, in1=Qe[:],
                                             op0=op.mult, op1=op.add)

                    dve.memset(esel[:, 0:1], 1.0)

                    Wown = W[:, :, 1:65, 1:17]
                    W8own = W[:, 1:KN, 1:65, 1:17]
                    pTv = prod[:].transpose([0, 3, 1, 2])
                    scb = scv[:].unsqueeze(1).to_broadcast([128, 3, H, YS])
                    Pb = P[:].unsqueeze(1).to_broadcast([128, 3, H, YS])
                    for ox in (-1, 0, 1):
                        if ox == 0:
                            Wsrc, Qsrc = W, Qe
                        else:
                            Wsrc, Qsrc = wr, Qr
                            if ox == -1:
                                nc.sync.dma_start(out=wr[1:128], in_=W[0:127])
                                nc.sync.dma_start(out=wr[0:1], in_=W[127:128])
                                nc.sync.dma_start(out=Qr[1:128], in_=Qe[0:127])
                                nc.sync.dma_start(out=Qr[0:1], in_=Qe[127:128])
                            else:
                                nc.sync.dma_start(out=wr[0:127], in_=W[1:128])
                                nc.sync.dma_start(out=wr[127:128], in_=W[0:1])
                                nc.sync.dma_start(out=Qr[0:127], in_=Qe[1:128])
                                nc.sync.dma_start(out=Qr[127:128], in_=Qe[0:1])
                        for oz in (-1, 0, 1):
                            for oy in (-1, 0, 1):
                                d = (oz + 1) * 9 + (oy + 1) * 3 + (ox + 1)
                                Wnb = Wsrc[:, :, 1 + oz:65 + oz,
                                           1 + oy:17 + oy]
                                dve.tensor_tensor(out=pTv, in0=Wown,
                                                  in1=Wnb, op=op.mult)
                                dve.tensor_reduce(out=Db[:, oy + 1],
                                                  in_=prod[:],
                                                  axis=mybir.AxisListType.X,
                                                  op=op.add)
                            # logit chain; the stt must stay <=3D canonical,
                            # so it runs per oy lane
                            for oy in (-1, 0, 1):
                                Qnb = Qsrc[:, 1 + oz:65 + oz, 1 + oy:17 + oy]
                                dve.scalar_tensor_tensor(
                                    out=Db[:, oy + 1], in0=Db[:, oy + 1],
                                    scalar=-2.0, in1=Qnb,
                                    op0=op.mult, op1=op.add)
                            dve.tensor_tensor(out=Db[:], in0=Db[:], in1=scb,
                                              op=op.mult)
                            dve.tensor_tensor(out=Db[:], in0=Db[:], in1=Pb,
                                              op=op.add)
                            act.activation(out=etb, in_=Db[:], func=AF.Exp)
                            for oy in (-1, 0, 1):
                                if oz == 0 and oy == 0 and ox == 0:
                                    continue
                                xnb = X3[:, ox + 1, 1 + oz:65 + oz,
                                         2 + oy:18 + oy]
                                dve.tensor_tensor(
                                    out=m8[:], in0=W8own,
                                    in1=xnb.unsqueeze(1).to_broadcast(
                                        [128, 8, H, YS]), op=op.is_equal)
                                dve.select(esel[:, 1:KN], m8[:],
                                           etb[:, oy + 1].unsqueeze(
                                               1).to_broadcast(
                                               [128, 8, H, YS]),
                                           esel[:, 1:KN])

                # softmax normalize + output
                with tc.tile_pool(name="fp", bufs=1) as fp:
                    ob = fp.tile([128, H, YS, KN], f32, tag="ob")
                    Ssum = fp.tile([128, H, YS], f32, tag="Ssum")
                    rec2 = fp.tile([128, H, YS], f32, tag="rec2")
                    obTv = ob[:].transpose([0, 3, 1, 2])
                    act.activation(out=obTv, in_=esel[:], func=AF.Copy)
                    dve.tensor_reduce(out=Ssum[:], in_=ob[:],
                                      axis=mybir.AxisListType.X, op=op.add)
                    dve.reciprocal(out=rec2[:], in_=Ssum[:])
                    dve.tensor_tensor(
                        out=obTv, in0=obTv,
                        in1=rec2[:].unsqueeze(1).to_broadcast(
                            [128, KN, H, YS]), op=op.mult)
                    nc.sync.dma_start(out=outd[:], in_=ob[:])

    nc.compile()
    return nc


# --------------------------------------------------------------------------
# Host side
# --------------------------------------------------------------------------

_CACHED = {}


def _get_nc(ks_value):
    key = float(ks_value)
    if key not in _CACHED:
        _CACHED[key] = build_bass(key)
    return _CACHED[key]


def _shard_inputs(x):
    """x: [H, M, N] f32 -> list of per-core xin arrays [128, 3, ZE, YI]."""
    maps = []
    zext = np.arange(-1, H + 1) % H
    xs = np.arange(N)
    for c in range(NCORES):
        ys = (np.arange(YS * c - 2, YS * c + YS + 2)) % M
        slab = x[zext][:, ys, :]                       # [66, 20, 128]
        a = np.empty((128, 3, ZE, YI), dtype=np.float32)
        for r in range(3):
            xrot = (xs + r - 1) % N
            a[:, r] = slab[:, :, xrot].transpose(2, 0, 1)
        maps.append({"xin": np.ascontiguousarray(a)})
    return maps


def kernel(input, ksigma, k, w):
    from concourse.bass_utils import run_bass_kernel_spmd

    x = np.asarray(input, dtype=np.float32)
    assert x.shape == (H, M, N)
    ks = float(np.asarray(ksigma).reshape(-1)[0])
    assert int(k) == KN and int(w) == 3

    nc = _get_nc(ks)
    in_maps = _shard_inputs(x)
    res = run_bass_kernel_spmd(nc, in_maps, core_ids=list(range(NCORES)))
    full = np.empty((H, M, N, KN), dtype=np.float32)
    for c in range(NCORES):
        oc = res.results[c]["out"]          # [128, H, YS, KN]
        full[:, YS * c:YS * c + YS] = oc.transpose(1, 2, 0, 3)
    return full.reshape(H * M * N, KN)


if __name__ == "__main__":
    nk = sum(1 for o in PLAN_OPS if o[0] == "keyg")
    nce = sum(int(o[3]) + int(o[4]) for o in PLAN_OPS if o[0] == "ce")
    print("plan: key-group ops", nk, "network min/max", nce)


# revision 5
# speedup vs baseline: 4.2407x; 4.2407x over previous
"""Trainium2 Bass kernel for nn_BuildK — instruction-count-minimal rewrite.

Shard y across 8 cores (no cross-core comms). Per core, x=128 on partitions,
single full-volume chunk (z=64, y=16+2 halo). Selection of the 9
intensity-nearest of 27 neighbors uses exact f32 |diff| keys through a
liveness-pruned top-8 sorting network; sorted values / per-rank neighbor
weights are recovered with equality masks + broadcast APs. Multi-lane
(overlapping-stride) views batch the oy-neighbor triples into single
instructions wherever the emulator's per-instruction cost dominates.
"""

import sys

sys.path.insert(0, "/opt/trn_rl_repo")

import numpy as np

H, M, N = 64, 128, 128
NCORES = 8
YS = M // NCORES          # 16 owned y rows per core
YE = YS + 2               # 18 = ext region (owned + 1 halo each side)
YI = YS + 4               # 20 = input slab y extent (halo 2)
ZE = H + 2                # 66 = z extent with periodic wrap rows
KN = 9
EPS = 1e-6
NSLOT = 22                # wire slots for the selection network
FD = H * YE               # 1152 ext-region elems

OFFS = [(oz, oy, ox) for oz in (-1, 0, 1) for oy in (-1, 0, 1)
        for ox in (-1, 0, 1)]            # reference enumeration; 13 = center

# wire -> candidate d, arranged so each sorter group is fed by oy-runs
# (fixed oz, ox; d stride 3) that become multi-lane views:
#   sorter A (wires 0-8):  oz=-1 block, 3 triples
#   sorter B (wires 9-17): oz=+1 block, 3 triples
#   sorter C (wires 18-25): oz=0 block, 2 triples + the (oy=+-1, ox=0) pair
WIRE_CAND = [0, 3, 6, 1, 4, 7, 2, 5, 8,
             18, 21, 24, 19, 22, 25, 20, 23, 26,
             9, 12, 15, 11, 14, 17, 10, 16]
KEY_RUNS = [(0, 3, 0), (3, 3, 1), (6, 3, 2),
            (9, 3, 18), (12, 3, 19), (15, 3, 20),
            (18, 3, 9), (21, 3, 11), (24, 2, 10)]  # (wire0, n, d0)
D_RUNS = [(3, 0), (3, 1), (3, 2), (3, 18), (3, 19), (3, 20),
          (3, 9), (3, 11), (2, 10)]                # (n, d0) for recon


def _lane_stride(d0, n):
    return 2 if (n == 2) else 1   # the pair run is oy=-1,+1 (step 2)


# --------------------------------------------------------------------------
# Selection network: sorted top-8 of the 26 non-center candidates.
# --------------------------------------------------------------------------

_SORT9 = [(0, 3), (1, 7), (2, 5), (4, 8), (0, 7), (2, 4), (3, 8), (5, 6),
          (0, 2), (1, 3), (4, 5), (7, 8), (1, 4), (3, 6), (5, 7), (0, 1),
          (2, 4), (3, 5), (6, 8), (2, 3), (4, 5), (6, 7), (1, 2), (3, 4),
          (5, 6)]

_S8 = [(0, 1), (2, 3), (4, 5), (6, 7), (0, 2), (1, 3), (4, 6), (5, 7),
       (1, 2), (5, 6), (0, 4), (3, 7), (1, 5), (2, 6), (1, 4), (3, 6),
       (2, 4), (3, 5), (3, 4)]


def _oddeven_merge(lo, n, r, out):
    step = r * 2
    if step < n:
        _oddeven_merge(lo, n, step, out)
        _oddeven_merge(lo + r, n, step, out)
        for i in range(lo + r, lo + n - r, step):
            out.append((i, i + r))
    else:
        out.append((lo, lo + r))


def _merge_topk(lenA, lenB, k):
    ces = []
    _oddeven_merge(0, 32, 1, ces)
    inf = [False] * 32
    for w in range(lenA, 16):
        inf[w] = True
    for w in range(16 + lenB, 32):
        inf[w] = True
    label = list(range(32))
    kept = []
    for (i, j) in ces:
        if inf[i] and inf[j]:
            continue
        if inf[j] and not inf[i]:
            continue
        if inf[i] and not inf[j]:
            label[i], label[j] = label[j], label[i]
            inf[i], inf[j] = False, True
            continue
        kept.append((label[i], label[j]))
    needed = set(label[w] for w in range(k))
    keep = []
    for (i, j) in reversed(kept):
        if i in needed or j in needed:
            keep.append((i, j))
            needed.add(i)
            needed.add(j)
    keep.reverse()

    def rm(w):
        return w if w < 16 else w - 16 + lenA

    return [(rm(i), rm(j)) for (i, j) in keep], [rm(label[w]) for w in range(k)]


def build_plan():
    """Ordered ops: ("keyg", run_idx) | ("ce", i, j, ni, nj); outw; lastw."""
    raw = []
    raw += [("keyg", 0), ("keyg", 1), ("keyg", 2)]
    raw += [("ce", i, j) for (i, j) in _SORT9]
    raw += [("keyg", 3), ("keyg", 4), ("keyg", 5)]
    raw += [("ce", i + 9, j + 9) for (i, j) in _SORT9]
    m1, ow1 = _merge_topk(9, 9, 8)
    raw += [("ce", i, j) for (i, j) in m1]
    raw += [("keyg", 6), ("keyg", 7), ("keyg", 8)]
    raw += [("ce", i + 18, j + 18) for (i, j) in _S8]
    m2, ow2 = _merge_topk(8, 8, 8)
    remap = {i: ow1[i] for i in range(8)}
    remap.update({8 + i: 18 + i for i in range(8)})
    raw += [("ce", remap[i], remap[j]) for (i, j) in m2]
    outw = [remap[w] for w in ow2]

    live = set(outw)
    ops = []
    for op_ in reversed(raw):
        if op_[0] == "keyg":
            ops.append(op_)
            continue
        _, i, j = op_
        ni, nj = i in live, j in live
        if not (ni or nj):
            continue
        ops.append(("ce", i, j, ni, nj))
        live.add(i)
        live.add(j)
    ops.reverse()

    lastw = {}
    for t, op_ in enumerate(ops):
        if op_[0] == "keyg":
            w0, n, _ = KEY_RUNS[op_[1]]
            for w in range(w0, w0 + n):
                lastw[w] = t
        else:
            _, i, j, ni, nj = op_
            if ni:
                lastw[i] = t
            if nj:
                lastw[j] = t
    for r, w in enumerate(outw):
        assert PLANCHK(ops, lastw, w), "output wire last-written by key op"
    return ops, outw, lastw


def PLANCHK(ops, lastw, w):
    return ops[lastw[w]][0] == "ce"


PLAN_OPS, PLAN_OUTW, PLAN_LASTW = build_plan()


# --------------------------------------------------------------------------
# Bass graph
# --------------------------------------------------------------------------

def build_bass(ks_value: float, reps: int = 1):
    import bass_rust
    from concourse import bacc, mybir
    from concourse import tile
    from concourse.alu_op_type import AluOpType as op
    from concourse.bass import MemorySpace

    f32 = mybir.dt.float32
    f16 = mybir.dt.float16
    u8 = mybir.dt.uint8
    AF = mybir.ActivationFunctionType

    nc = bacc.Bacc("TRN2", target_bir_lowering=False, debug=False,
                   num_devices=NCORES)

    xin = nc.dram_tensor("xin", [128, 3, ZE, YI], f32, kind="ExternalInput").ap()
    outd = nc.dram_tensor("out", [128, H, YS, KN], f32,
                          kind="ExternalOutput").ap()

    dve = nc.vector
    act = nc.scalar

    rank_of = {w: r for r, w in enumerate(PLAN_OUTW)}  # wire -> K8 column

    def lanes(ap3, n, stride):
        """Insert a leading free dim [n, stride] into a [128, a, b] AP."""
        b = ap3.copy()
        pairs = [list(p) for p in ap3.ap]
        b.ap = bass_rust.VecI64Pair([pairs[0], [stride, n]] + pairs[1:])
        return b

    with tile.TileContext(nc) as tc:
      for _rep in range(reps):
        with tc.tile_pool(name="pp", bufs=1) as pp:
            X3 = pp.tile([128, 3, ZE, YI], f32, tag="X3")
            W = pp.tile([128, KN, ZE, YE], f32, tag="W")
            nc.sync.dma_start(out=X3[:], in_=xin[:])

            # ext-region views (z rows 0..63, y -1..16)
            def vview(d):
                oz, oy, ox = OFFS[d]
                return X3[:, ox + 1, 1 + oz:65 + oz, 1 + oy:19 + oy]

            def vrun(d0, n):
                """Multi-lane view: lanes l=0..n-1 are candidates d0+3l
                (same oz/ox, oy ascending)."""
                stride = _lane_stride(d0, n)
                return lanes(vview(d0), n, stride)

            cv = X3[:, 1, 1:65, 1:19]

            with tc.tile_pool(name="kp", bufs=1) as kp:
                K8 = kp.tile([128, 8, H, YE], f32, tag="K8")

                # ------------- keys + selection network -------------
                with tc.tile_pool(name="sp", bufs=1) as sp:
                    kbig = sp.tile([128, NSLOT, FD], f32, tag="kbig")
                    free_slots = list(range(NSLOT))
                    wire_ap = {}
                    wire_slot = {}

                    def alloc_ap(wire, t):
                        if t == PLAN_LASTW[wire] and wire in rank_of:
                            return K8[:, rank_of[wire]], None
                        s = free_slots.pop()
                        return kbig[:, s, :], s

                    def alloc_run(n):
                        ss = sorted(free_slots)
                        for i in range(len(ss) - n + 1):
                            if ss[i + n - 1] == ss[i] + n - 1:
                                for s in range(ss[i], ss[i] + n):
                                    free_slots.remove(s)
                                return ss[i]
                        raise RuntimeError("no contiguous slot run")

                    for t, op_ in enumerate(PLAN_OPS):
                        if op_[0] == "keyg":
                            w0, n, d0 = KEY_RUNS[op_[1]]
                            s0 = alloc_run(n)
                            kv = kbig[:, s0:s0 + n, :]
                            dve.tensor_tensor(
                                out=kv, in0=vrun(d0, n),
                                in1=cv.unsqueeze(1).to_broadcast(
                                    [128, n, H, YE]), op=op.subtract)
                            dve.scalar_tensor_tensor(
                                out=kv, in0=kv, scalar=-1.0, in1=kv,
                                op0=op.mult, op1=op.max)
                            for l in range(n):
                                wire_ap[w0 + l] = kbig[:, s0 + l, :]
                                wire_slot[w0 + l] = s0 + l
                            continue
                        _, i, j, ni, nj = op_
                        ai, aj = wire_ap[i], wire_ap[j]
                        si, sj = wire_slot[i], wire_slot[j]
                        if ni:
                            new_ai, new_si = alloc_ap(i, t)
                            dve.tensor_tensor(out=new_ai, in0=ai, in1=aj,
                                              op=op.min)
                        if nj:
                            new_aj, new_sj = alloc_ap(j, t)
                            dve.tensor_tensor(out=new_aj, in0=ai, in1=aj,
                                              op=op.max)
                        if si is not None:
                            free_slots.append(si)
                        if sj is not None:
                            free_slots.append(sj)
                        if ni:
                            wire_ap[i], wire_slot[i] = new_ai, new_si
                        else:
                            del wire_ap[i], wire_slot[i]
                        if nj:
                            wire_ap[j], wire_slot[j] = new_aj, new_sj
                        else:
                            del wire_ap[j], wire_slot[j]

                # ------------- W reconstruction (equality masks) ------------
                with tc.tile_pool(name="cp", bufs=1) as cp:
                    ktg = cp.tile([128, 3, H, YE], f32, tag="ktg")
                    mg = cp.tile([128, 3, 8, FD], u8, tag="mg")
                    K8f = K8[:].rearrange("p r a b -> p r (a b)")
                    for (n, d0) in D_RUNS:
                        kv = ktg[:, 0:n]
                        dve.tensor_tensor(
                            out=kv, in0=vrun(d0, n),
                            in1=cv.unsqueeze(1).to_broadcast(
                                [128, n, H, YE]), op=op.subtract)
                        dve.scalar_tensor_tensor(
                            out=kv, in0=kv, scalar=-1.0, in1=kv,
                            op0=op.mult, op1=op.max)
                        kvf = kv.rearrange("p n a b -> p n (a b)")
                        dve.tensor_tensor(
                            out=mg[:, 0:n],
                            in0=K8f.unsqueeze(1).to_broadcast(
                                [128, n, 8, FD]),
                            in1=kvf.unsqueeze(2).to_broadcast(
                                [128, n, 8, FD]),
                            op=op.is_equal)
                        stride = _lane_stride(d0, n)
                        for l in range(n):
                            d = d0 + 3 * stride * l
                            dve.select(W[:, 1:KN, 1:65, :], mg[:, l],
                                       vview(d).unsqueeze(1).to_broadcast(
                                           [128, 8, H, YE]),
                                       W[:, 1:KN, 1:65, :])
                    act.activation(out=W[:, 0, 1:65, :], in_=cv, func=AF.Copy)

            # z wrap rows of W
            nc.sync.dma_start(out=W[:, :, 0:1, :], in_=W[:, :, 64:65, :])
            nc.sync.dma_start(out=W[:, :, 65:66, :], in_=W[:, :, 1:2, :])

            # ------------- dots + softmax -------------
            with tc.tile_pool(name="dp", bufs=1) as dp:
                # Qe/Qr rows 1..66 hold Q for zw 0..65; rows 0/67 are pad so
                # the 3-lane (oy) neighbor view canonicalizes to 3 dims
                wr = dp.tile([128, KN, ZE, YE], f32, tag="wr")
                Qe = dp.tile([128, ZE + 2, YE], f32, tag="Qe")
                Qr = dp.tile([128, ZE + 2, YE], f32, tag="Qr")
                Se = dp.tile([128, ZE, YE], f32, tag="Se")
                P = dp.tile([128, H, YE], f32, tag="P")
                scv = dp.tile([128, H, YE], f32, tag="scv")
                esel = dp.tile([128, KN, H, YS], f16, tag="esel")

                def qlanes(Qt, oz):
                    b = Qt[:].copy()
                    b.ap = bass_rust.VecI64Pair(
                        [list(Qt[:].ap)[0], [1, 3], [YE, H], [1, YE]])
                    b.offset = (2 + oz) * YE - 1
                    return b

                with tc.tile_pool(name="dp1", bufs=1) as dp1:
                    prod = dp1.tile([128, H, YS, KN], f32, tag="prod")
                    m8 = dp1.tile([128, 8, H, YS], u8, tag="m8")
                    Db = dp1.tile([128, 3, H, YE], f32, tag="Db")
                    # etb aliases the front of prod (free there once the
                    # last reduce of a row has consumed prod)
                    etb = prod[:].rearrange("p a b r -> p (a b r)")[
                        :, 0:3 * H * YE].rearrange(
                        "p (l a b) -> p l a b", l=3, a=H, b=YE)

                    # Q (sum of squares) and S (sum) over ranks, ext region,
                    # chunked through the owned-size prod scratch
                    pflat = prod[:].rearrange("p a b r -> p (a b) r")
                    for h in range(2):
                        zlo = 33 * h
                        sl = 33 * YE
                        pv = pflat[:, 0:sl, :].transpose([0, 2, 1])
                        wv = W[:, :, zlo:zlo + 33, :].rearrange(
                            "p r a b -> p r (a b)")
                        qv = Qe[:, 1 + zlo:34 + zlo, :].rearrange(
                            "p a b -> p (a b)")
                        sv = Se[:, zlo:zlo + 33, :].rearrange(
                            "p a b -> p (a b)")
                        dve.tensor_tensor(out=pv, in0=wv, in1=wv, op=op.mult)
                        dve.tensor_reduce(out=qv, in_=pflat[:, 0:sl, :],
                                          axis=mybir.AxisListType.X,
                                          op=op.add)
                        act.activation(out=pv, in_=wv, func=AF.Copy)
                        dve.tensor_reduce(out=sv, in_=pflat[:, 0:sl, :],
                                          axis=mybir.AxisListType.X,
                                          op=op.add)

                    # sigma / scale planes (owned region); etb = scratch
                    Qo = Qe[:, 2:66, 1:17]
                    S1 = Se[:, 1:65, 1:17]
                    ta, tb = etb[:, 0, :, 1:17], etb[:, 1, :, 1:17]
                    act.activation(out=ta, in_=S1, func=AF.Square)
                    dve.scalar_tensor_tensor(out=ta, in0=ta,
                                             scalar=-1.0 / 9.0, in1=Qo,
                                             op0=op.mult, op1=op.add)
                    dve.tensor_scalar(out=tb, in0=ta, scalar1=0.0,
                                      scalar2=None, op0=op.is_equal)
                    dve.tensor_tensor(out=tb, in0=tb, in1=ta, op=op.add)
                    dve.reciprocal(out=scv[:, :, 1:17], in_=tb)
                    dve.tensor_scalar(out=scv[:, :, 1:17],
                                      in0=scv[:, :, 1:17],
                                      scalar1=-4.0 / (ks_value * ks_value),
                                      scalar2=None, op0=op.mult)
                    dve.tensor_scalar(out=tb, in0=ta, scalar1=0.0,
                                      scalar2=None, op0=op.not_equal)
                    dve.tensor_tensor(out=scv[:, :, 1:17],
                                      in0=scv[:, :, 1:17], in1=tb,
                                      op=op.mult)
                    dve.scalar_tensor_tensor(out=P[:, :, 1:17], in0=S1,
                                             scalar=2.0 * EPS, in1=Qo,
                                             op0=op.mult, op1=op.add)
                    dve.tensor_scalar(out=P[:, :, 1:17], in0=P[:, :, 1:17],
                                      scalar1=9.0 * EPS * EPS, scalar2=None,
                                      op0=op.add)
                    dve.tensor_tensor(out=P[:, :, 1:17], in0=P[:, :, 1:17],
                                      in1=scv[:, :, 1:17], op=op.mult)
                    # fold eps into neighbor plane: Qe <- Qe - 2 eps Se
                    dve.scalar_tensor_tensor(out=Qe[:, 1:67, :], in0=Se[:],
                                             scalar=-2.0 * EPS,
                                             in1=Qe[:, 1:67, :],
                                             op0=op.mult, op1=op.add)

                    dve.memset(esel[:, 0:1], 1.0)

                    Wown = W[:, :, 1:65, 1:17]
                    W8own = W[:, 1:KN, 1:65, 1:17]
                    pTv = prod[:].transpose([0, 3, 1, 2])
                    scb = scv[:].unsqueeze(1).to_broadcast([128, 3, H, YE])
                    Pb = P[:].unsqueeze(1).to_broadcast([128, 3, H, YE])
                    for ox in (-1, 0, 1):
                        if ox == 0:
                            Wsrc, Qsrc = W, Qe
                        else:
                            Wsrc, Qsrc = wr, Qr
                            if ox == -1:
                                nc.sync.dma_start(out=wr[1:128], in_=W[0:127])
                                nc.sync.dma_start(out=wr[0:1], in_=W[127:128])
                                nc.sync.dma_start(out=Qr[1:128], in_=Qe[0:127])
                                nc.sync.dma_start(out=Qr[0:1], in_=Qe[127:128])
                            else:
                                nc.sync.dma_start(out=wr[0:127], in_=W[1:128])
                                nc.sync.dma_start(out=wr[127:128], in_=W[0:1])
                                nc.sync.dma_start(out=Qr[0:127], in_=Qe[1:128])
                                nc.sync.dma_start(out=Qr[127:128], in_=Qe[0:1])
                        for oz in (-1, 0, 1):
                            for oy in (-1, 0, 1):
                                d = (oz + 1) * 9 + (oy + 1) * 3 + (ox + 1)
                                Wnb = Wsrc[:, :, 1 + oz:65 + oz,
                                           1 + oy:17 + oy]
                                dve.tensor_tensor(out=pTv, in0=Wown,
                                                  in1=Wnb, op=op.mult)
                                dve.tensor_reduce(
                                    out=Db[:, oy + 1, :, 1:17], in_=prod[:],
                                    axis=mybir.AxisListType.X, op=op.add)
                            # batched logit chain: padded-Q lane view
                            # canonicalizes to 3 dims
                            dve.scalar_tensor_tensor(
                                out=Db[:], in0=Db[:], scalar=-2.0,
                                in1=qlanes(Qsrc, oz), op0=op.mult,
                                op1=op.add)
                            dve.tensor_tensor(out=Db[:], in0=Db[:], in1=scb,
                                              op=op.mult)
                            dve.tensor_tensor(out=Db[:], in0=Db[:], in1=Pb,
                                              op=op.add)
                            act.activation(out=etb, in_=Db[:], func=AF.Exp)
                            for oy in (-1, 0, 1):
                                if oz == 0 and oy == 0 and ox == 0:
                                    continue
                                xnb = X3[:, ox + 1, 1 + oz:65 + oz,
                                         2 + oy:18 + oy]
                                dve.tensor_tensor(
                                    out=m8[:], in0=W8own,
                                    in1=xnb.unsqueeze(1).to_broadcast(
                                        [128, 8, H, YS]), op=op.is_equal)
                                dve.select(esel[:, 1:KN], m8[:],
                                           etb[:, oy + 1, :, 1:17].unsqueeze(
                                               1).to_broadcast(
                                               [128, 8, H, YS]),
                                           esel[:, 1:KN])

                # softmax normalize + output
                with tc.tile_pool(name="fp", bufs=1) as fp:
                    ob = fp.tile([128, H, YS, KN], f32, tag="ob")
                    Ssum = fp.tile([128, H, YS], f32, tag="Ssum")
                    rec2 = fp.tile([128, H, YS], f32, tag="rec2")
                    obTv = ob[:].transpose([0, 3, 1, 2])
                    act.activation(out=obTv, in_=esel[:], func=AF.Copy)
                    dve.tensor_reduce(out=Ssum[:], in_=ob[:],
                                      axis=mybir.AxisListType.X, op=op.add)
                    dve.reciprocal(out=rec2[:], in_=Ssum[:])
                    dve.tensor_tensor(
                        out=obTv, in0=obTv,
                        in1=rec2[:].unsqueeze(1).to_broadcast(
                            [128, KN, H, YS]), op=op.mult)
                    nc.sync.dma_start(out=outd[:], in_=ob[:])

    nc.compile()
    return nc


# --------------------------------------------------------------------------
# Host side
# --------------------------------------------------------------------------

_CACHED = {}


def _get_nc(ks_value):
    key = float(ks_value)
    if key not in _CACHED:
        _CACHED[key] = build_bass(key)
    return _CACHED[key]


def _shard_inputs(x):
    """x: [H, M, N] f32 -> list of per-core xin arrays [128, 3, ZE, YI]."""
    maps = []
    zext = np.arange(-1, H + 1) % H
    xs = np.arange(N)
    for c in range(NCORES):
        ys = (np.arange(YS * c - 2, YS * c + YS + 2)) % M
        slab = x[zext][:, ys, :]                       # [66, 20, 128]
        a = np.empty((128, 3, ZE, YI), dtype=np.float32)
        for r in range(3):
            xrot = (xs + r - 1) % N
            a[:, r] = slab[:, :, xrot].transpose(2, 0, 1)
        maps.append({"xin": np.ascontiguousarray(a)})
    return maps


def kernel(input, ksigma, k, w):
    from concourse.bass_utils import run_bass_kernel_spmd

    x = np.asarray(input, dtype=np.float32)
    assert x.shape == (H, M, N)
    ks = float(np.asarray(ksigma).reshape(-1)[0])
    assert int(k) == KN and int(w) == 3

    nc = _get_nc(ks)
    in_maps = _shard_inputs(x)
    res = run_bass_kernel_spmd(nc, in_maps, core_ids=list(range(NCORES)))
    full = np.empty((H, M, N, KN), dtype=np.float32)
    for c in range(NCORES):
        oc = res.results[c]["out"]          # [128, H, YS, KN]
        full[:, YS * c:YS * c + YS] = oc.transpose(1, 2, 0, 3)
    return full.reshape(H * M * N, KN)


if __name__ == "__main__":
    nk = sum(1 for o in PLAN_OPS if o[0] == "keyg")
    nce = sum(int(o[3]) + int(o[4]) for o in PLAN_OPS if o[0] == "ce")
    print("plan: key-group ops", nk, "network min/max", nce)


# revision 6
# speedup vs baseline: 8.1385x; 1.9192x over previous
"""Trainium2 Bass kernel for nn_BuildK — instruction-count-minimal rewrite.

Shard y across 8 cores (no cross-core comms). Per core, x=128 on partitions,
single full-volume chunk (z=64, y=16+2 halo). Selection of the 9
intensity-nearest of 27 neighbors uses exact f32 |diff| keys through a
liveness-pruned top-8 sorting network; sorted values / per-rank neighbor
weights are recovered with equality masks + broadcast APs. Multi-lane
(overlapping-stride) views batch the oy-neighbor triples into single
instructions wherever the emulator's per-instruction cost dominates.
"""

import sys

sys.path.insert(0, "/opt/trn_rl_repo")

import numpy as np

H, M, N = 64, 128, 128
NCORES = 8
YS = M // NCORES          # 16 owned y rows per core
YE = YS + 2               # 18 = ext region (owned + 1 halo each side)
YI = YS + 4               # 20 = input slab y extent (halo 2)
ZE = H + 2                # 66 = z extent with periodic wrap rows
KN = 9
EPS = 1e-6
NSLOT = 22                # wire slots for the selection network
FD = H * YE               # 1152 ext-region elems

OFFS = [(oz, oy, ox) for oz in (-1, 0, 1) for oy in (-1, 0, 1)
        for ox in (-1, 0, 1)]            # reference enumeration; 13 = center

# wire -> candidate d, arranged so each sorter group is fed by oy-runs
# (fixed oz, ox; d stride 3) that become multi-lane views:
#   sorter A (wires 0-8):  oz=-1 block, 3 triples
#   sorter B (wires 9-17): oz=+1 block, 3 triples
#   sorter C (wires 18-25): oz=0 block, 2 triples + the (oy=+-1, ox=0) pair
WIRE_CAND = [0, 3, 6, 1, 4, 7, 2, 5, 8,
             18, 21, 24, 19, 22, 25, 20, 23, 26,
             9, 12, 15, 11, 14, 17, 10, 16]
KEY_RUNS = [(0, 3, 0), (3, 3, 1), (6, 3, 2),
            (9, 3, 18), (12, 3, 19), (15, 3, 20),
            (18, 3, 9), (21, 3, 11), (24, 2, 10)]  # (wire0, n, d0)
D_RUNS = [(3, 0), (3, 1), (3, 2), (3, 18), (3, 19), (3, 20),
          (3, 9), (3, 11), (2, 10)]                # (n, d0) for recon


def _lane_stride(d0, n):
    return 2 if (n == 2) else 1   # the pair run is oy=-1,+1 (step 2)


# --------------------------------------------------------------------------
# Selection network: sorted top-8 of the 26 non-center candidates.
# --------------------------------------------------------------------------

_SORT9 = [(0, 3), (1, 7), (2, 5), (4, 8), (0, 7), (2, 4), (3, 8), (5, 6),
          (0, 2), (1, 3), (4, 5), (7, 8), (1, 4), (3, 6), (5, 7), (0, 1),
          (2, 4), (3, 5), (6, 8), (2, 3), (4, 5), (6, 7), (1, 2), (3, 4),
          (5, 6)]

_S8 = [(0, 1), (2, 3), (4, 5), (6, 7), (0, 2), (1, 3), (4, 6), (5, 7),
       (1, 2), (5, 6), (0, 4), (3, 7), (1, 5), (2, 6), (1, 4), (3, 6),
       (2, 4), (3, 5), (3, 4)]


def _oddeven_merge(lo, n, r, out):
    step = r * 2
    if step < n:
        _oddeven_merge(lo, n, step, out)
        _oddeven_merge(lo + r, n, step, out)
        for i in range(lo + r, lo + n - r, step):
            out.append((i, i + r))
    else:
        out.append((lo, lo + r))


def _merge_topk(lenA, lenB, k):
    ces = []
    _oddeven_merge(0, 32, 1, ces)
    inf = [False] * 32
    for w in range(lenA, 16):
        inf[w] = True
    for w in range(16 + lenB, 32):
        inf[w] = True
    label = list(range(32))
    kept = []
    for (i, j) in ces:
        if inf[i] and inf[j]:
            continue
        if inf[j] and not inf[i]:
            continue
        if inf[i] and not inf[j]:
            label[i], label[j] = label[j], label[i]
            inf[i], inf[j] = False, True
            continue
        kept.append((label[i], label[j]))
    needed = set(label[w] for w in range(k))
    keep = []
    for (i, j) in reversed(kept):
        if i in needed or j in needed:
            keep.append((i, j))
            needed.add(i)
            needed.add(j)
    keep.reverse()

    def rm(w):
        return w if w < 16 else w - 16 + lenA

    return [(rm(i), rm(j)) for (i, j) in keep], [rm(label[w]) for w in range(k)]


def build_plan():
    """Ordered ops: ("keyg", run_idx) | ("ce", i, j, ni, nj); outw; lastw."""
    raw = []
    raw += [("keyg", i) for i in range(6)]
    raw += [("sortab",)]
    m1, ow1 = _merge_topk(9, 9, 8)
    raw += [("ce", i, j) for (i, j) in m1]
    raw += [("keyg", 6), ("keyg", 7), ("keyg", 8)]
    raw += [("ce", i + 18, j + 18) for (i, j) in _S8]
    m2, ow2 = _merge_topk(8, 8, 8)
    remap = {i: ow1[i] for i in range(8)}
    remap.update({8 + i: 18 + i for i in range(8)})
    raw += [("ce", remap[i], remap[j]) for (i, j) in m2]
    outw = [remap[w] for w in ow2]

    live = set(outw)
    ops = []
    for op_ in reversed(raw):
        if op_[0] in ("keyg", "sortab"):
            ops.append(op_)
            if op_[0] == "sortab":
                live.update(range(18))
            continue
        _, i, j = op_
        ni, nj = i in live, j in live
        if not (ni or nj):
            continue
        ops.append(("ce", i, j, ni, nj))
        live.add(i)
        live.add(j)
    ops.reverse()

    lastw = {}
    for t, op_ in enumerate(ops):
        if op_[0] == "keyg":
            w0, n, _ = KEY_RUNS[op_[1]]
            for w in range(w0, w0 + n):
                lastw[w] = t
        elif op_[0] == "sortab":
            for w in range(18):
                lastw[w] = t
        else:
            _, i, j, ni, nj = op_
            if ni:
                lastw[i] = t
            if nj:
                lastw[j] = t
    for r, w in enumerate(outw):
        assert PLANCHK(ops, lastw, w), "output wire last-written by key op"
    return ops, outw, lastw


def PLANCHK(ops, lastw, w):
    return ops[lastw[w]][0] == "ce"


PLAN_OPS, PLAN_OUTW, PLAN_LASTW = build_plan()


# --------------------------------------------------------------------------
# Bass graph
# --------------------------------------------------------------------------

def build_bass(ks_value: float, reps: int = 1):
    import bass_rust
    from concourse import bacc, mybir
    from concourse import tile
    from concourse.alu_op_type import AluOpType as op
    from concourse.bass import MemorySpace

    f32 = mybir.dt.float32
    f16 = mybir.dt.float16
    u8 = mybir.dt.uint8
    AF = mybir.ActivationFunctionType

    nc = bacc.Bacc("TRN2", target_bir_lowering=False, debug=False,
                   num_devices=NCORES)

    xin = nc.dram_tensor("xin", [128, 3, ZE, YI], f32, kind="ExternalInput").ap()
    outd = nc.dram_tensor("out", [128, H, YS, KN], f32,
                          kind="ExternalOutput").ap()

    dve = nc.vector
    act = nc.scalar

    rank_of = {w: r for r, w in enumerate(PLAN_OUTW)}  # wire -> K8 column

    def lanes(ap3, n, stride):
        """Insert a leading free dim [n, stride] into a [128, a, b] AP."""
        b = ap3.copy()
        pairs = [list(p) for p in ap3.ap]
        b.ap = bass_rust.VecI64Pair([pairs[0], [stride, n]] + pairs[1:])
        return b

    with tile.TileContext(nc) as tc:
      for _rep in range(reps):
        with tc.tile_pool(name="pp", bufs=1) as pp:
            X3 = pp.tile([128, 3, ZE, YI], f32, tag="X3")
            W = pp.tile([128, KN, ZE, YE], f32, tag="W")
            nc.sync.dma_start(out=X3[:], in_=xin[:])

            # ext-region views (z rows 0..63, y -1..16)
            def vview(d):
                oz, oy, ox = OFFS[d]
                return X3[:, ox + 1, 1 + oz:65 + oz, 1 + oy:19 + oy]

            def vrun(d0, n):
                """Multi-lane view: lanes l=0..n-1 are candidates d0+3l
                (same oz/ox, oy ascending)."""
                stride = _lane_stride(d0, n)
                return lanes(vview(d0), n, stride)

            cv = X3[:, 1, 1:65, 1:19]

            with tc.tile_pool(name="kp", bufs=1) as kp:
                K8 = kp.tile([128, 8, H, YE], f32, tag="K8")

                # ------------- keys + selection network -------------
                with tc.tile_pool(name="sp", bufs=1) as sp:
                    kbig = sp.tile([128, NSLOT, FD], f32, tag="kbig")
                    free_slots = list(range(NSLOT))
                    wire_ap = {}
                    wire_slot = {}

                    def alloc_ap(wire, t):
                        if t == PLAN_LASTW[wire] and wire in rank_of:
                            return K8[:, rank_of[wire]], None
                        s = free_slots.pop()
                        return kbig[:, s, :], s

                    def alloc_run(n):
                        ss = sorted(free_slots)
                        for i in range(len(ss) - n + 1):
                            if ss[i + n - 1] == ss[i] + n - 1:
                                for s in range(ss[i], ss[i] + n):
                                    free_slots.remove(s)
                                return ss[i]
                        raise RuntimeError("no contiguous slot run")

                    BANK = 11
                    for t, op_ in enumerate(PLAN_OPS):
                        if op_[0] == "sortab":
                            # sort9 on groups A and B in lockstep: each
                            # min/max is one 2-lane op (bank stride 11)
                            for ss in (9, 10, 20, 21):
                                free_slots.remove(ss)
                            pos = {w: w for w in range(9)}
                            freep = [9, 10]

                            def pv(pp_):
                                return lanes(kbig[:, pp_, :], 2, BANK * FD)

                            for (i, j) in _SORT9:
                                ps = freep.pop()
                                dve.tensor_tensor(out=pv(ps),
                                                  in0=pv(pos[i]),
                                                  in1=pv(pos[j]), op=op.min)
                                dve.tensor_tensor(out=pv(pos[j]),
                                                  in0=pv(pos[i]),
                                                  in1=pv(pos[j]), op=op.max)
                                freep.append(pos[i])
                                pos[i] = ps
                            for w in range(9):
                                wire_ap[w] = kbig[:, pos[w], :]
                                wire_slot[w] = pos[w]
                                wire_ap[9 + w] = kbig[:, BANK + pos[w], :]
                                wire_slot[9 + w] = BANK + pos[w]
                            for fp_ in freep:
                                free_slots.append(fp_)
                                free_slots.append(BANK + fp_)
                            continue
                        if op_[0] == "keyg":
                            w0, n, d0 = KEY_RUNS[op_[1]]
                            if w0 < 18:
                                s0 = w0 if w0 < 9 else w0 + 2
                                for ss in range(s0, s0 + n):
                                    free_slots.remove(ss)
                            else:
                                s0 = alloc_run(n)
                            kv = kbig[:, s0:s0 + n, :]
                            dve.tensor_tensor(
                                out=kv, in0=vrun(d0, n),
                                in1=cv.unsqueeze(1).to_broadcast(
                                    [128, n, H, YE]), op=op.subtract)
                            dve.scalar_tensor_tensor(
                                out=kv, in0=kv, scalar=-1.0, in1=kv,
                                op0=op.mult, op1=op.max)
                            for l in range(n):
                                wire_ap[w0 + l] = kbig[:, s0 + l, :]
                                wire_slot[w0 + l] = s0 + l
                            continue
                        _, i, j, ni, nj = op_
                        ai, aj = wire_ap[i], wire_ap[j]
                        si, sj = wire_slot[i], wire_slot[j]
                        if ni:
                            new_ai, new_si = alloc_ap(i, t)
                            dve.tensor_tensor(out=new_ai, in0=ai, in1=aj,
                                              op=op.min)
                        if nj:
                            new_aj, new_sj = alloc_ap(j, t)
                            dve.tensor_tensor(out=new_aj, in0=ai, in1=aj,
                                              op=op.max)
                        if si is not None:
                            free_slots.append(si)
                        if sj is not None:
                            free_slots.append(sj)
                        if ni:
                            wire_ap[i], wire_slot[i] = new_ai, new_si
                        else:
                            del wire_ap[i], wire_slot[i]
                        if nj:
                            wire_ap[j], wire_slot[j] = new_aj, new_sj
                        else:
                            del wire_ap[j], wire_slot[j]

                # ------------- W reconstruction (equality masks) ------------
                with tc.tile_pool(name="cp", bufs=1) as cp:
                    ktg = cp.tile([128, 3, H, YE], f32, tag="ktg")
                    mg = cp.tile([128, 3, 8, FD], u8, tag="mg")
                    K8f = K8[:].rearrange("p r a b -> p r (a b)")
                    for (n, d0) in D_RUNS:
                        kv = ktg[:, 0:n]
                        dve.tensor_tensor(
                            out=kv, in0=vrun(d0, n),
                            in1=cv.unsqueeze(1).to_broadcast(
                                [128, n, H, YE]), op=op.subtract)
                        dve.scalar_tensor_tensor(
                            out=kv, in0=kv, scalar=-1.0, in1=kv,
                            op0=op.mult, op1=op.max)
                        kvf = kv.rearrange("p n a b -> p n (a b)")
                        dve.tensor_tensor(
                            out=mg[:, 0:n],
                            in0=K8f.unsqueeze(1).to_broadcast(
                                [128, n, 8, FD]),
                            in1=kvf.unsqueeze(2).to_broadcast(
                                [128, n, 8, FD]),
                            op=op.is_equal)
                        stride = _lane_stride(d0, n)
                        for l in range(n):
                            d = d0 + 3 * stride * l
                            dve.select(W[:, 1:KN, 1:65, :], mg[:, l],
                                       vview(d).unsqueeze(1).to_broadcast(
                                           [128, 8, H, YE]),
                                       W[:, 1:KN, 1:65, :])
                    act.activation(out=W[:, 0, 1:65, :], in_=cv, func=AF.Copy)

            # z wrap rows of W
            nc.sync.dma_start(out=W[:, :, 0:1, :], in_=W[:, :, 64:65, :])
            nc.sync.dma_start(out=W[:, :, 65:66, :], in_=W[:, :, 1:2, :])

            # ------------- dots + softmax -------------
            with tc.tile_pool(name="dp", bufs=1) as dp:
                # Qe/Qr rows 1..66 hold Q for zw 0..65; rows 0/67 are pad so
                # the 3-lane (oy) neighbor view canonicalizes to 3 dims
                wr = dp.tile([128, KN, ZE, YE], f32, tag="wr")
                Qe = dp.tile([128, ZE + 2, YE], f32, tag="Qe")
                Qr = dp.tile([128, ZE + 2, YE], f32, tag="Qr")
                Se = dp.tile([128, ZE, YE], f32, tag="Se")
                P = dp.tile([128, H, YE], f32, tag="P")
                scv = dp.tile([128, H, YE], f32, tag="scv")
                esel = dp.tile([128, KN, H, YS], f16, tag="esel")

                def qlanes(Qt, oz):
                    b = Qt[:].copy()
                    b.ap = bass_rust.VecI64Pair(
                        [list(Qt[:].ap)[0], [1, 3], [YE, H], [1, YE]])
                    b.offset = (2 + oz) * YE - 1
                    return b

                with tc.tile_pool(name="dp1", bufs=1) as dp1:
                    prod = dp1.tile([128, H, YS, KN], f32, tag="prod")
                    m8 = dp1.tile([128, 8, H, YS], u8, tag="m8")
                    Db = dp1.tile([128, 3, H, YE], f32, tag="Db")
                    # etb aliases the front of prod (free there once the
                    # last reduce of a row has consumed prod)
                    etb = prod[:].rearrange("p a b r -> p (a b r)")[
                        :, 0:3 * H * YE].rearrange(
                        "p (l a b) -> p l a b", l=3, a=H, b=YE)

                    # Q (sum of squares) and S (sum) over ranks, ext region,
                    # chunked through the owned-size prod scratch
                    pflat = prod[:].rearrange("p a b r -> p (a b) r")
                    for h in range(2):
                        zlo = 33 * h
                        sl = 33 * YE
                        pv = pflat[:, 0:sl, :].transpose([0, 2, 1])
                        wv = W[:, :, zlo:zlo + 33, :].rearrange(
                            "p r a b -> p r (a b)")
                        qv = Qe[:, 1 + zlo:34 + zlo, :].rearrange(
                            "p a b -> p (a b)")
                        sv = Se[:, zlo:zlo + 33, :].rearrange(
                            "p a b -> p (a b)")
                        dve.tensor_tensor(out=pv, in0=wv, in1=wv, op=op.mult)
                        dve.tensor_reduce(out=qv, in_=pflat[:, 0:sl, :],
                                          axis=mybir.AxisListType.X,
                                          op=op.add)
                        act.activation(out=pv, in_=wv, func=AF.Copy)
                        dve.tensor_reduce(out=sv, in_=pflat[:, 0:sl, :],
                                          axis=mybir.AxisListType.X,
                                          op=op.add)

                    # sigma / scale planes (owned region); etb = scratch
                    Qo = Qe[:, 2:66, 1:17]
                    S1 = Se[:, 1:65, 1:17]
                    ta, tb = etb[:, 0, :, 1:17], etb[:, 1, :, 1:17]
                    act.activation(out=ta, in_=S1, func=AF.Square)
                    dve.scalar_tensor_tensor(out=ta, in0=ta,
                                             scalar=-1.0 / 9.0, in1=Qo,
                                             op0=op.mult, op1=op.add)
                    dve.tensor_scalar(out=tb, in0=ta, scalar1=0.0,
                                      scalar2=None, op0=op.is_equal)
                    dve.tensor_tensor(out=tb, in0=tb, in1=ta, op=op.add)
                    dve.reciprocal(out=scv[:, :, 1:17], in_=tb)
                    dve.tensor_scalar(out=scv[:, :, 1:17],
                                      in0=scv[:, :, 1:17],
                                      scalar1=-4.0 / (ks_value * ks_value),
                                      scalar2=None, op0=op.mult)
                    dve.tensor_scalar(out=tb, in0=ta, scalar1=0.0,
                                      scalar2=None, op0=op.not_equal)
                    dve.tensor_tensor(out=scv[:, :, 1:17],
                                      in0=scv[:, :, 1:17], in1=tb,
                                      op=op.mult)
                    dve.scalar_tensor_tensor(out=P[:, :, 1:17], in0=S1,
                                             scalar=2.0 * EPS, in1=Qo,
                                             op0=op.mult, op1=op.add)
                    dve.tensor_scalar(out=P[:, :, 1:17], in0=P[:, :, 1:17],
                                      scalar1=9.0 * EPS * EPS, scalar2=None,
                                      op0=op.add)
                    dve.tensor_tensor(out=P[:, :, 1:17], in0=P[:, :, 1:17],
                                      in1=scv[:, :, 1:17], op=op.mult)
                    # fold eps into neighbor plane: Qe <- Qe - 2 eps Se
                    dve.scalar_tensor_tensor(out=Qe[:, 1:67, :], in0=Se[:],
                                             scalar=-2.0 * EPS,
                                             in1=Qe[:, 1:67, :],
                                             op0=op.mult, op1=op.add)

                    dve.memset(esel[:, 0:1], 1.0)

                    Wown = W[:, :, 1:65, 1:17]
                    W8own = W[:, 1:KN, 1:65, 1:17]
                    pTv = prod[:].transpose([0, 3, 1, 2])
                    scb = scv[:].unsqueeze(1).to_broadcast([128, 3, H, YE])
                    Pb = P[:].unsqueeze(1).to_broadcast([128, 3, H, YE])
                    for ox in (-1, 0, 1):
                        if ox == 0:
                            Wsrc, Qsrc = W, Qe
                        else:
                            Wsrc, Qsrc = wr, Qr
                            if ox == -1:
                                nc.sync.dma_start(out=wr[1:128], in_=W[0:127])
                                nc.sync.dma_start(out=wr[0:1], in_=W[127:128])
                                nc.sync.dma_start(out=Qr[1:128], in_=Qe[0:127])
                                nc.sync.dma_start(out=Qr[0:1], in_=Qe[127:128])
                            else:
                                nc.sync.dma_start(out=wr[0:127], in_=W[1:128])
                                nc.sync.dma_start(out=wr[127:128], in_=W[0:1])
                                nc.sync.dma_start(out=Qr[0:127], in_=Qe[1:128])
                                nc.sync.dma_start(out=Qr[127:128], in_=Qe[0:1])
                        for oz in (-1, 0, 1):
                            for oy in (-1, 0, 1):
                                d = (oz + 1) * 9 + (oy + 1) * 3 + (ox + 1)
                                Wnb = Wsrc[:, :, 1 + oz:65 + oz,
                                           1 + oy:17 + oy]
                                dve.tensor_tensor(out=pTv, in0=Wown,
                                                  in1=Wnb, op=op.mult)
                                dve.tensor_reduce(
                                    out=Db[:, oy + 1, :, 1:17], in_=prod[:],
                                    axis=mybir.AxisListType.X, op=op.add)
                            # batched logit chain: padded-Q lane view
                            # canonicalizes to 3 dims
                            dve.scalar_tensor_tensor(
                                out=Db[:], in0=Db[:], scalar=-2.0,
                                in1=qlanes(Qsrc, oz), op0=op.mult,
                                op1=op.add)
                            dve.tensor_tensor(out=Db[:], in0=Db[:], in1=scb,
                                              op=op.mult)
                            dve.tensor_tensor(out=Db[:], in0=Db[:], in1=Pb,
                                              op=op.add)
                            act.activation(out=etb, in_=Db[:], func=AF.Exp)
                            for oy in (-1, 0, 1):
                                if oz == 0 and oy == 0 and ox == 0:
                                    continue
                                xnb = X3[:, ox + 1, 1 + oz:65 + oz,
                                         2 + oy:18 + oy]
                                dve.tensor_tensor(
                                    out=m8[:], in0=W8own,
                                    in1=xnb.unsqueeze(1).to_broadcast(
                                        [128, 8, H, YS]), op=op.is_equal)
                                dve.select(esel[:, 1:KN], m8[:],
                                           etb[:, oy + 1, :, 1:17].unsqueeze(
                                               1).to_broadcast(
                                               [128, 8, H, YS]),
                                           esel[:, 1:KN])

                # softmax normalize + output
                with tc.tile_pool(name="fp", bufs=1) as fp:
                    ob = fp.tile([128, H, YS, KN], f32, tag="ob")
                    Ssum = fp.tile([128, H, YS], f32, tag="Ssum")
                    rec2 = fp.tile([128, H, YS], f32, tag="rec2")
                    obTv = ob[:].transpose([0, 3, 1, 2])
                    act.activation(out=obTv, in_=esel[:], func=AF.Copy)
                    dve.tensor_reduce(out=Ssum[:], in_=ob[:],
                                      axis=mybir.AxisListType.X, op=op.add)
                    dve.reciprocal(out=rec2[:], in_=Ssum[:])
                    dve.tensor_tensor(
                        out=obTv, in0=obTv,
                        in1=rec2[:].unsqueeze(1).to_broadcast(
                            [128, KN, H, YS]), op=op.mult)
                    nc.sync.dma_start(out=outd[:], in_=ob[:])

    nc.compile()
    return nc


# --------------------------------------------------------------------------
# Host side
# --------------------------------------------------------------------------

_CACHED = {}


def _get_nc(ks_value):
    key = float(ks_value)
    if key not in _CACHED:
        _CACHED[key] = build_bass(key)
    return _CACHED[key]


def _shard_inputs(x):
    """x: [H, M, N] f32 -> list of per-core xin arrays [128, 3, ZE, YI]."""
    maps = []
    zext = np.arange(-1, H + 1) % H
    xs = np.arange(N)
    for c in range(NCORES):
        ys = (np.arange(YS * c - 2, YS * c + YS + 2)) % M
        slab = x[zext][:, ys, :]                       # [66, 20, 128]
        a = np.empty((128, 3, ZE, YI), dtype=np.float32)
        for r in range(3):
            xrot = (xs + r - 1) % N
            a[:, r] = slab[:, :, xrot].transpose(2, 0, 1)
        maps.append({"xin": np.ascontiguousarray(a)})
    return maps


def kernel(input, ksigma, k, w):
    from concourse.bass_utils import run_bass_kernel_spmd

    x = np.asarray(input, dtype=np.float32)
    assert x.shape == (H, M, N)
    ks = float(np.asarray(ksigma).reshape(-1)[0])
    assert int(k) == KN and int(w) == 3

    nc = _get_nc(ks)
    in_maps = _shard_inputs(x)
    res = run_bass_kernel_spmd(nc, in_maps, core_ids=list(range(NCORES)))
    full = np.empty((H, M, N, KN), dtype=np.float32)
    for c in range(NCORES):
        oc = res.results[c]["out"]          # [128, H, YS, KN]
        full[:, YS * c:YS * c + YS] = oc.transpose(1, 2, 0, 3)
    return full.reshape(H * M * N, KN)


if __name__ == "__main__":
    nk = sum(1 for o in PLAN_OPS if o[0] == "keyg")
    nce = sum(int(o[3]) + int(o[4]) for o in PLAN_OPS if o[0] == "ce")
    print("plan: key-group ops", nk, "network min/max", nce)
